# revision 36
# baseline (speedup 1.0000x reference)
"""Bass/Trainium2 kernel for nn_AttentionLayer_68229850464552.

Full multi-head causal attention layer (QKV proj + partial RoPE + attention +
output proj), head-sharded (tensor parallel) across 8 NeuronCores. Each core
computes 2 of the 16 heads for both batch elements and the partial output
projection for its heads' feature columns; the host sums the 8 partials and
adds the output bias.

Matmul operands are bf16 (PE streams 2B/lane/cycle -> 1 cycle/row; fp32/fp32r
stream at half rate); accumulation is fp32 in PSUM throughout.

v2 changes vs baseline:
 - host pre-lays-out x/weights in the exact SBUF tile layouts (contiguous
   DMA descriptors, no strided rearranges)
 - first chunk + first weight M-tile DMA split per k-tile so the first QKV
   matmul starts after ~160KB instead of ~6MB
 - qk/v tile pools double-buffered so batch 1's QKV projection overlaps
   batch 0's attention drain (removes the batch-boundary PE bubble + the
   HAM cold-clock window it caused)
 - partial out-proj written to HBM in bf16 (halves output DMA)
 - RoPE element-wise ops in bf16 (2x DVE mode)
 - softmax normalization reads otps/sums directly from PSUM (drops two
   PSUM->SBUF copies and the gpsimd mult per unit)

Self-contained: hardcodes shapes from the problem spec.
"""
import os
import numpy as np
import ml_dtypes
from contextlib import ExitStack

import concourse.bass as bass
import concourse.mybir as mybir
import concourse.tile as tile
from concourse import bacc
from concourse.bass_utils import run_bass_kernel_spmd

B, S, D, H, DK = 2, 2048, 2048, 16, 128
HPC = 2                      # heads per core
NCORES = 8
DR = 32                      # rope features
SCALE = 1.0 / float(np.sqrt(DK))
CH = 512                     # x seq-chunk width for the QKV projection
NCH = S // CH                # 4
QCW = 512                    # query chunk width in attention
NQC = S // QCW               # 4
NJ = S // 128                # 16 key blocks
WQ_COLS = 4 * 128            # q0,q1,k0,k1 M-tiles
WV_COLS = 2 * 129            # [v_h0 | ones] [v_h1 | ones]

F32 = mybir.dt.float32
BF16 = mybir.dt.bfloat16
Act = mybir.ActivationFunctionType
Alu = mybir.AluOpType
BF_NP = ml_dtypes.bfloat16

_PROG_CACHE = {}


def _build_program():
    nc = bacc.Bacc("TRN2", target_bir_lowering=False, debug=False,
                   enable_asserts=True, num_devices=NCORES)

    # host-side layouts match the SBUF tile layouts exactly (contiguous DMA)
    xH = nc.dram_tensor("xH", [B, NCH, 128, 16, CH], BF16,
                        kind="ExternalInput").ap()
    x00 = nc.dram_tensor("x00", [2, 128, 16, 256], BF16,
                         kind="ExternalInput").ap()
    wq = nc.dram_tensor("wq", [4, 128, 16, 128], BF16,
                        kind="ExternalInput").ap()
    wv = nc.dram_tensor("wv", [128, 16, WV_COLS], BF16,
                        kind="ExternalInput").ap()
    wo = nc.dram_tensor("wo", [128, 2, D], BF16, kind="ExternalInput").ap()
    bqk = nc.dram_tensor("bqk", [128, 4], F32, kind="ExternalInput").ap()
    bv = nc.dram_tensor("bv", [128, WV_COLS], F32, kind="ExternalInput").ap()
    cosT = nc.dram_tensor("cosT", [DR, S], BF16, kind="ExternalInput").ap()
    sinT = nc.dram_tensor("sinT", [DR, S], BF16, kind="ExternalInput").ap()
    maskT = nc.dram_tensor("maskT", [128, 128], BF16, kind="ExternalInput").ap()
    idm = nc.dram_tensor("idm", [128, 128], BF16, kind="ExternalInput").ap()
    pout = nc.dram_tensor("pout", [B * S // 128, 128, D], BF16,
                          kind="ExternalOutput").ap()

    with tile.TileContext(nc) as tc, ExitStack() as ctx:
        wpool = ctx.enter_context(tc.tile_pool(name="w", bufs=1))
        xpool = ctx.enter_context(tc.tile_pool(name="x", bufs=3))
        qkpool = ctx.enter_context(tc.tile_pool(name="qk", bufs=2))
        vpool = ctx.enter_context(tc.tile_pool(name="v", bufs=2))
        otpool = ctx.enter_context(tc.tile_pool(name="ot", bufs=1))
        ppool = ctx.enter_context(tc.tile_pool(name="p", bufs=3))
        rpool = ctx.enter_context(tc.tile_pool(name="r", bufs=3))
        opool = ctx.enter_context(tc.tile_pool(name="o", bufs=3))
        scpool = ctx.enter_context(tc.tile_pool(name="sc", bufs=2, space="PSUM"))
        accpool = ctx.enter_context(tc.tile_pool(name="acc", bufs=4, space="PSUM"))
        pjpool = ctx.enter_context(tc.tile_pool(name="pj", bufs=2, space="PSUM"))

        # resident weights / constants. DMA queue order matters: small early
        # deps first, then first weight M-tile / first x chunk interleaved
        # per k-tile, then the rest in order of first use. b=0 chunks are
        # prefetched early so the QKV stream is never DMA-starved.
        cos_sb = wpool.tile([DR, S], BF16)
        nc.sync.dma_start(cos_sb[:], cosT[:])
        sin_sb = wpool.tile([DR, S], BF16)
        nc.sync.dma_start(sin_sb[:], sinT[:])
        bqk_sb = wpool.tile([128, 4], F32)
        nc.sync.dma_start(bqk_sb[:], bqk[:])
        wq_sb = wpool.tile([128, 4, 16, 128], BF16)
        x00_sb = wpool.tile([128, 2, 16, 256], BF16)
        nc.sync.dma_start(wq_sb[:, 0], wq[0])
        for sub in range(2):
            nc.sync.dma_start(x00_sb[:, sub], x00[sub])
        for mt in range(1, 4):
            nc.sync.dma_start(wq_sb[:, mt], wq[mt])
        xt_b0 = {}
        xt_b0[1] = xpool.tile([128, 16, CH], BF16, tag="xt", name="xt_c1")
        nc.sync.dma_start(xt_b0[1][:], xH[0, 1])
        wv_sb = wpool.tile([128, 16, WV_COLS], BF16)
        nc.sync.dma_start(wv_sb[:], wv[:])
        bv_sb = wpool.tile([128, WV_COLS], F32)
        nc.sync.dma_start(bv_sb[:], bv[:])
        xt_b0[2] = xpool.tile([128, 16, CH], BF16, tag="xt", name="xt_c2")
        nc.sync.dma_start(xt_b0[2][:], xH[0, 2])
        maskT_sb = wpool.tile([128, 128], BF16)
        nc.sync.dma_start(maskT_sb[:], maskT[:])
        idm_sb = wpool.tile([128, 128], BF16)
        nc.sync.dma_start(idm_sb[:], idm[:])
        ones_sb = wpool.tile([128, 128], BF16)
        nc.gpsimd.memset(ones_sb[:], 1.0)
        xt_b0[3] = xpool.tile([128, 16, CH], BF16, tag="xt", name="xt_c3")
        nc.sync.dma_start(xt_b0[3][:], xH[0, 3])
        wo_sb = wpool.tile([128, 2, D], BF16)
        nc.sync.dma_start(wo_sb[:], wo[:])

        # warm the PE HAM clock-gate with junk matmuls while the first real
        # operands stream in (only dep: cos_sb, 128KB, first in DMA queue)
        warm_ps = scpool.tile([128, 512], F32, tag="sc", name="warm")
        for _ in range(7):
            nc.tensor.matmul(warm_ps[:], cos_sb[:, 0:128], cos_sb[:, 0:512],
                             start=True, stop=True)

        def outproj_fn(b, ot_sb):
            def _outproj(qc=NQC - 1, half=None):
                if half is None:
                    sblks = range(4 * qc, 4 * qc + 4)
                elif half == 0:
                    sblks = range(4 * qc, 4 * qc + 2)
                else:
                    sblks = range(4 * qc + 2, 4 * qc + 4)
                for sblk in sblks:
                    po = opool.tile([128, D], BF16, tag="po", name="po")
                    for n in range(D // 512):
                        ps = pjpool.tile([128, 512], F32, tag="pj",
                                         name="psC")
                        for kt in range(2):
                            nc.tensor.matmul(
                                ps[:],
                                ot_sb[:, kt, sblk * 128:(sblk + 1) * 128],
                                wo_sb[:, kt, n * 512:(n + 1) * 512],
                                start=(kt == 0), stop=(kt == 1))
                        if (sblk + n) % 2 == 0:
                            nc.vector.tensor_copy(
                                po[:, n * 512:(n + 1) * 512], ps[:])
                        else:
                            nc.scalar.activation(
                                po[:, n * 512:(n + 1) * 512], ps[:], Act.Copy)
                    nc.sync.dma_start(pout[b * (S // 128) + sblk], po[:])
            return _outproj

        pending = []
        prefetched = None
        for b in range(B):
            # ---------------- Phase A: QKV projection + RoPE ----------------
            # qk_sb[t]: [feat(128), S] for t in (q_h0, q_h1, k_h0, k_h1)
            qk_sb = [qkpool.tile([128, S], BF16, tag=f"qk{t}", name=f"qk{t}")
                     for t in range(4)]
            v_sb = vpool.tile([128, NJ, WV_COLS], BF16, tag="v")

            for c in range(NCH):
                cs = slice(c * CH, (c + 1) * CH)
                if b == 0 and c == 0:
                    # sub-chunk startup path: first matmul only needs
                    # wq[0] + x00[0] (~1.5MB) instead of the full 2.5MB
                    for sub in range(2):
                        for mt in range(4):
                            ps = pjpool.tile([128, CH], F32, tag="pj")
                            for kt in range(16):
                                nc.tensor.matmul(
                                    ps[:, 0:256], wq_sb[:, mt, kt, :],
                                    x00_sb[:, sub, kt, :],
                                    start=(kt == 0), stop=(kt == 15))
                            nc.scalar.activation(
                                qk_sb[mt][:, sub * 256:(sub + 1) * 256],
                                ps[:, 0:256],
                                Act.Identity, bias=bqk_sb[:, mt:mt + 1])
                        for s2 in range(2):
                            psv = pjpool.tile([128, WV_COLS], F32, tag="pj")
                            for kt in range(16):
                                nc.tensor.matmul(
                                    psv[:],
                                    x00_sb[:, sub, kt,
                                           s2 * 128:(s2 + 1) * 128],
                                    wv_sb[:, kt, :], start=(kt == 0),
                                    stop=(kt == 15))
                            nc.vector.tensor_tensor(
                                v_sb[:, sub * 2 + s2, :], psv[:], bv_sb[:],
                                Alu.add)
                    for t4 in range(4):
                        shuf = rpool.tile([DR, CH], BF16, tag="shuf",
                                          name="shuf")
                        nc.sync.dma_start(shuf[0:16, :], qk_sb[t4][16:32, cs])
                        nc.sync.dma_start(shuf[16:32, :], qk_sb[t4][0:16, cs])
                        tmp = rpool.tile([DR, CH], BF16, tag="rt", name="tmp")
                        nc.vector.tensor_tensor(tmp[:], shuf[:],
                                                sin_sb[:, cs], Alu.mult)
                        tgt = qk_sb[t4][0:DR, cs]
                        nc.vector.tensor_tensor(tgt, tgt, cos_sb[:, cs],
                                                Alu.mult)
                        nc.vector.tensor_tensor(tgt, tgt, tmp[:], Alu.add)
                    if pending:
                        pending.pop(0)()
                    continue
                if b == 0:
                    xt = xt_b0[c]
                elif c == 0:
                    xt = prefetched
                else:
                    xt = xpool.tile([128, 16, CH], BF16, tag="xt")
                    nc.sync.dma_start(xt[:], xH[b, c])

                for mt in range(4):
                    ps = pjpool.tile([128, CH], F32, tag="pj")
                    for kt in range(16):
                        nc.tensor.matmul(
                            ps[:], wq_sb[:, mt, kt, :],
                            xt[:, kt, :], start=(kt == 0), stop=(kt == 15))
                    nc.scalar.activation(qk_sb[mt][:, cs], ps[:],
                                         Act.Identity,
                                         bias=bqk_sb[:, mt:mt + 1])

                # RoPE on the first DR features of each q/k tensor, per chunk:
                # rot = [q[16:32] (sign folded into sinT), q[0:16]]
                for t4 in range(4):
                    shuf = rpool.tile([DR, CH], BF16, tag="shuf", name="shuf")
                    nc.sync.dma_start(shuf[0:16, :], qk_sb[t4][16:32, cs])
                    nc.sync.dma_start(shuf[16:32, :], qk_sb[t4][0:16, cs])
                    tmp = rpool.tile([DR, CH], BF16, tag="rt", name="tmp")
                    nc.vector.tensor_tensor(tmp[:], shuf[:], sin_sb[:, cs],
                                            Alu.mult)
                    tgt = qk_sb[t4][0:DR, cs]
                    nc.vector.tensor_tensor(tgt, tgt, cos_sb[:, cs], Alu.mult)
                    nc.vector.tensor_tensor(tgt, tgt, tmp[:], Alu.add)

                # V projection for this chunk ([seq, feat] layout, + ones col)
                for s2 in range(CH // 128):
                    psv = pjpool.tile([128, WV_COLS], F32, tag="pj")
                    for kt in range(16):
                        nc.tensor.matmul(
                            psv[:], xt[:, kt, s2 * 128:(s2 + 1) * 128],
                            wv_sb[:, kt, :], start=(kt == 0), stop=(kt == 15))
                    nc.vector.tensor_tensor(
                        v_sb[:, c * (CH // 128) + s2, :], psv[:],
                        bv_sb[:], Alu.add)

                if c == 0 and pending:
                    pending.pop(0)()

            # prefetch next batch's first x chunk during attention
            if b + 1 < B:
                xt_next = xpool.tile([128, 16, CH], BF16, tag="xt",
                                     name="xt_next")
                nc.sync.dma_start(xt_next[:], xH[b + 1, 0])
            else:
                xt_next = None

            # ------- Phase B + C: attention, pipelined with out-proj --------
            ot_sb = otpool.tile([128, HPC, S], BF16, tag="ot")
            rsums = otpool.tile([128, NQC * HPC, QCW], F32, tag="rsm")

            def norm_h(qc, h, otps, sums):
                i_qh = qc * HPC + h
                nc.vector.reciprocal_approx_fast(rsums[:, i_qh, :], sums[:])
                nc.vector.tensor_tensor(
                    ot_sb[:, h, qc * QCW:(qc + 1) * QCW],
                    otps[:], rsums[:, i_qh, :], Alu.mult)

            def outproj(qc, half):
                outproj_fn(b, ot_sb)(qc, half)

            for qc in range(NQC):
                jmax = 4 * qc + 3
                for h in range(HPC):
                    otps = accpool.tile([128, QCW], F32, tag="acc")
                    sums = accpool.tile([128, QCW], F32, tag="acc")

                    def emit_score(j):
                        c0 = (j - 4 * qc) * 128 if j >= 4 * qc else 0
                        diag = j >= 4 * qc
                        sps = scpool.tile([128, QCW], F32, tag="sc",
                                          name="sps")
                        nc.tensor.matmul(
                            sps[:, c0:QCW], qk_sb[2 + h][:, j * 128:(j + 1) * 128],
                            qk_sb[h][:, qc * QCW + c0:(qc + 1) * QCW],
                            start=True, stop=not diag)
                        if diag:
                            # add -1e4 above the diagonal of the diag subblock
                            nc.tensor.matmul(
                                sps[:, c0:c0 + 128], maskT_sb[:], idm_sb[:],
                                start=False, stop=True)
                        return sps

                    def emit_consume(j, sps):
                        c0 = (j - 4 * qc) * 128 if j >= 4 * qc else 0
                        pt = ppool.tile([128, QCW], BF16, tag="pt", name="pt")
                        nc.scalar.activation(pt[:, c0:QCW], sps[:, c0:QCW],
                                             Act.Exp, scale=SCALE)
                        nc.tensor.matmul(
                            otps[:, c0:QCW],
                            v_sb[:, j, 129 * h:129 * h + 128],
                            pt[:, c0:QCW], start=(j == 0), stop=(j == jmax))
                        nc.tensor.matmul(
                            sums[:, c0:QCW], ones_sb[:],
                            pt[:, c0:QCW], start=(j == 0), stop=(j == jmax))

                    prev = emit_score(0)
                    for j in range(1, jmax + 1):
                        cur = emit_score(j)
                        emit_consume(j - 1, prev)
                        prev = cur
                    emit_consume(jmax, prev)
                    norm_h(qc, h, otps, sums)
                    if qc >= 1:
                        outproj(qc - 1, h)
            pending.append(outproj_fn(b, ot_sb))
            prefetched = xt_next
        while pending:
            pending.pop(0)()

    nc.compile()
    return nc


def kernel(x, W_qkv, b_qkv, W_out, b_out):
    x = np.asarray(x, dtype=np.float32)
    W_qkv = np.asarray(W_qkv, dtype=np.float32)
    b_qkv = np.asarray(b_qkv, dtype=np.float32)
    W_out = np.asarray(W_out, dtype=np.float32)
    b_out = np.asarray(b_out, dtype=np.float32)

    if "prog" not in _PROG_CACHE:
        _PROG_CACHE["prog"] = _build_program()
    nc = _PROG_CACHE["prog"]

    xT = x.transpose(0, 2, 1)                       # [B, D, S]
    xH = np.ascontiguousarray(
        xT.reshape(B, 16, 128, NCH, CH).transpose(0, 3, 2, 1, 4)
    ).astype(BF_NP)                                 # [B, NCH, 128, 16, CH]
    x00 = np.ascontiguousarray(
        np.asarray(xH[0, 0]).reshape(128, 16, 2, 256).transpose(2, 0, 1, 3))

    i = np.arange(16, dtype=np.float64)
    theta = 1.0 / (10000.0 ** ((2.0 * i) / DR))
    s_idx = np.arange(S, dtype=np.float64)
    idx = s_idx[:, None] * theta[None, :]          # [S, 16]
    idx2 = np.concatenate([idx, idx], axis=1)      # [S, 32]
    cosT = np.ascontiguousarray(np.cos(idx2).T.astype(np.float32))
    sinT = np.sin(idx2).T.astype(np.float32)
    sinT[0:16, :] *= -1.0          # sign of rot = [-q[16:32], q[0:16]] folded in
    sinT = np.ascontiguousarray(sinT)
    cosT = cosT.astype(BF_NP)
    sinT = sinT.astype(BF_NP)

    maskT = np.triu(np.full((128, 128), -10000.0, dtype=np.float32), 1).astype(BF_NP)
    idm = np.eye(128, dtype=np.float32).astype(BF_NP)

    in_maps = []
    for c in range(NCORES):
        heads = [HPC * c, HPC * c + 1]
        qw, kw, vw, qb, kb, vb = [], [], [], [], [], []
        for hh in heads:
            base = 3 * DK * hh
            qw.append(W_qkv[base:base + 128])
            kw.append(W_qkv[base + 128:base + 256])
            vw.append(W_qkv[base + 256:base + 384])
            qb.append(b_qkv[base:base + 128])
            kb.append(b_qkv[base + 128:base + 256])
            vb.append(b_qkv[base + 256:base + 384])

        M = np.concatenate([qw[0], qw[1], kw[0], kw[1]], axis=0)  # [512, D]
        # wq[mt, p, kt, j] = M[mt*128+j, kt*128+p]
        wq_np = np.ascontiguousarray(
            M.reshape(4, 128, 16, 128).transpose(0, 3, 2, 1)).astype(BF_NP)

        Mv = np.zeros((WV_COLS, D), dtype=np.float32)
        Mv[0:128] = vw[0]
        Mv[129:257] = vw[1]
        # wv[p, kt, m] = Mv[m, kt*128+p]
        wv_np = np.ascontiguousarray(
            Mv.T.reshape(16, 128, WV_COLS).transpose(1, 0, 2)).astype(BF_NP)

        bv_np = np.zeros((1, WV_COLS), dtype=np.float32)
        bv_np[0, 0:128] = vb[0]
        bv_np[0, 128] = 1.0
        bv_np[0, 129:257] = vb[1]
        bv_np[0, 257] = 1.0
        bv_np = np.ascontiguousarray(np.repeat(bv_np, 128, axis=0))

        bqk_np = np.zeros((128, 4), dtype=np.float32)
        bqk_np[:, 0] = qb[0]
        bqk_np[:, 1] = qb[1]
        bqk_np[:, 2] = kb[0]
        bqk_np[:, 3] = kb[1]

        Mo = np.ascontiguousarray(
            W_out[:, HPC * DK * c: HPC * DK * (c + 1)].T)       # [256, D]
        wo_np = np.ascontiguousarray(
            Mo.reshape(2, 128, D).transpose(1, 0, 2)).astype(BF_NP)

        in_maps.append({
            "xH": xH, "x00": x00, "wq": wq_np, "wv": wv_np, "wo": wo_np,
            "bqk": bqk_np, "bv": bv_np, "cosT": cosT, "sinT": sinT,
            "maskT": maskT, "idm": idm,
        })

    trace = os.environ.get("KERNEL_TRACE", "0") == "1"
    res = run_bass_kernel_spmd(nc, in_maps, core_ids=list(range(NCORES)),
                               trace=trace)
    if res.exec_time_ns is not None:
        print(f"HW exec time: {res.exec_time_ns} ns")
        if res.instructions_and_trace is not None:
            print(f"trace: {res.instructions_and_trace[1]}")

    acc = np.zeros((B * S, D), dtype=np.float64)
    for c in range(NCORES):
        acc += res.results[c]["pout"].reshape(B * S, D).astype(np.float64)
    out = (acc + b_out.astype(np.float64)[None, :]).astype(np.float32)
    return out.reshape(B, S, D)
